# revision 19
# baseline (speedup 1.0000x reference)
"""Trainium2 Bass kernel for nn_Attn_17059610099812.

reference:
    energies = einsum('sh,h->s', encoder_outputs[131072, 512], hidden[512])
    attn = softmax(energies)   -> [1, 1, 131072]

Strategy (8 NeuronCores, SPMD):
  - Shard encoder_outputs along seq_len: 16384 rows per core (host-side
    split via per-core input maps).
  - Per core: stream the 32MB shard in 4 slabs of 8MB (pn layout: each SBUF
    partition holds n=32 consecutive rows as one contiguous 64KB DMA
    descriptor), double-buffered across 2 HWDGE queues; DMA runs at ~430GB/s.
  - mode "segdot": per slab, ONE single-pass custom DVE instruction
    (multiply-cumsum: out[k] = sum_{i<=k} slab[i]*w[i], fp32 scan at
    1 elem/partition/cycle) computes running dot products in-place; a tiny
    strided copy samples column 511 of each 512-element page = cumulative
    row energies. The host recovers per-row energies by adjacent
    differences. DVE does exactly one pass over the data, so the kernel
    stays DMA-bound (~80-90us/core) with ~20 instructions total.
  - mode "ttr": stock two-pass fallback (tensor_mul + segmented
    tensor_reduce) — no custom op, ~125us/core.
  - Device ships raw energy samples (64KB/core); the host finishes with a
    float64 softmax over 131072 values (negligible) and the pn unpermute.

kernel() accepts the FULL inputs and returns the FULL [1, 1, 131072] output.
"""

import numpy as np

SEQ = 131072
H = 512
NCORES = 8
SHARD = SEQ // NCORES          # 16384 rows per core
NBLK = SHARD // 128            # 128 energies per partition

_CACHE = {}

BUILD_KW = {"n": 32, "bufs": 3, "queues": ("sync", "scalar"),
            "mode": "ramp", "w_via_pe": True}


def _register_mult_cumsum():
    """Process-local custom DVE op: out[p,k] = sum_{i<=k} in0[p,i]*in1[p,i].

    The framework writes per-NEFF DVE tables from the process-local op
    catalog; registering here (instead of editing dve_ops.py) is the
    self-contained equivalent of adding the op to the catalog. uops hashes
    are pinned from lower() output, same as test_ops_golden would print.
    """
    from concourse import dve_ops
    from concourse.dve_spec import AluOp, Spec, Src0, Src1, scan
    from concourse.dve_uop import DveOpSpec

    name = "ANT_MULT_CUMSUM"
    for op in dve_ops.OPS:
        if op.name == name:
            return op

    def _ref(in0, in1, c0, c1, c2):
        a = np.asarray(in0, dtype=np.float32)
        b = np.asarray(in1, dtype=np.float32)
        if b.shape != a.shape:
            b = np.broadcast_to(
                b.reshape(b.shape[0], -1), a.reshape(a.shape[0], -1).shape
            ).reshape(a.shape)
        prod = a * b
        p2 = prod.reshape(prod.shape[0], -1)
        return np.cumsum(p2, axis=-1, dtype=np.float32).reshape(prod.shape)

    spec = Spec(body=scan(AluOp.ADD, Src0 * Src1), reference=_ref)
    row = max(dve_ops._SUB_OPCODE_FOR_NAME.values()) + 1
    assert row < 0x20
    op = dve_ops.DveOp(name, spec, subdim=False, uops_sha={})
    dve_ops.OPS.append(op)
    dve_ops.CUSTOM_DVE_SPECS[name] = spec
    dve_ops._SUB_OPCODE_FOR_NAME[name] = row
    for ver in ("v3", "v4"):
        r = DveOpSpec(name=name, opcode=row, uops=dve_ops.lower(spec, ver=ver),
                      rd1_en=dve_ops.has_src1(spec))
        op.uops_sha[ver] = r.sha(ver)
    return op


SIZES_UNEVEN = (64, 32, 32)     # rows/partition per slab; sum == NBLK
# small slab first for an early scan start; sizes interleaved so the two
# HWDGE queues carry equal bytes (sync: 16+16+32, scalar: 32+32 rows/part)
SIZES_RAMP = (16, 32, 16, 32, 32)


def _build_program(n=32, bufs=2, queues=("sync", "scalar"), mode="segdot",
                   w_via_pe=False, _repeat=1):
    import concourse.bacc as bacc
    import concourse.mybir as mybir
    import concourse.tile as tile

    f32 = mybir.dt.float32
    Alu = mybir.AluOpType
    Ax = mybir.AxisListType

    if mode != "uneven":
        assert NBLK % n == 0
    if mode in ("segdot", "uneven", "ramp"):
        cumsum_op = _register_mult_cumsum()

    nc = bacc.Bacc(
        "TRN2", target_bir_lowering=False, debug=False, num_devices=NCORES
    )
    enc = nc.dram_tensor("enc", [SHARD, H], f32, kind="ExternalInput")
    hid = nc.dram_tensor("hid", [1, H], f32, kind="ExternalInput")
    eng_out = nc.dram_tensor("energies", [128, NBLK], f32,
                             kind="ExternalOutput")

    big_bufs = 1 if mode == "uneven" else (2 if mode == "ramp" else bufs)
    bigB_bufs = 2 if mode == "ramp" else 1
    with tile.TileContext(nc) as tc:
        with (
            tc.tile_pool(name="big", bufs=big_bufs) as big_pool,
            tc.tile_pool(name="small", bufs=1) as small_pool,
            tc.tile_pool(name="bigB", bufs=bigB_bufs) as bigB_pool,
        ):
            # w_sb[p, h] = hidden[h] on every partition
            w_sb = small_pool.tile([128, H], f32, tag="w")
            if w_via_pe:
                # ones-matmul broadcast: the hid DMA is a single descriptor
                # (vs 128 for a partition-broadcast DMA)
                with tc.tile_pool(name="psum", bufs=1, space="PSUM") as psum_p:
                    ones_t = small_pool.tile([1, 128], f32, tag="ones")
                    nc.vector.memset(ones_t[:], 1.0)
                    hid_sb = small_pool.tile([1, H], f32, tag="hid")
                    nc.sync.dma_start(hid_sb[:], hid[:])
                    w_ps = psum_p.tile([128, H], f32, tag="wps")
                    nc.tensor.matmul(w_ps[:], ones_t[:], hid_sb[:],
                                     start=True, stop=True)
                    nc.scalar.copy(w_sb[:], w_ps[:])
            else:
                nc.sync.dma_start(w_sb[:], hid.ap().broadcast_to([128, H]))

            # e_sb[p, o + j]: cumulative energy sample of page j in the slab
            # at block-offset o ("segdot"/"uneven"); plain energy for "ttr"
            e_sb = small_pool.tile([128, NBLK], f32, tag="e")

            def do_slab(slab, s3, src_ap, n_t, o, queue):
                w_b = w_sb[:].unsqueeze(1).broadcast_to([128, n_t, H])
                getattr(nc, queue).dma_start(slab[:], src_ap)
                if mode == "ttr":
                    nc.vector.tensor_mul(s3, s3, w_b)
                    nc.vector.tensor_reduce(
                        e_sb[:, o:o + n_t], s3, axis=Ax.X, op=Alu.add)
                else:
                    nc.vector._custom_dve(cumsum_op, out=s3, in0=s3, in1=w_b)
                    nc.vector.tensor_copy(
                        e_sb[:, o:o + n_t], s3[:, :, H - 1:H].squeeze(2))

            for rep in range(_repeat):
                if mode in ("uneven", "ramp"):
                    # uneven: slab A (big pool); the rest share bigB.
                    # ramp: small slabs (bigB) first for an early scan
                    # start, then big slabs (big pool).
                    sizes = SIZES_UNEVEN if mode == "uneven" else SIZES_RAMP
                    big_size = max(sizes)
                    o = 0
                    for i, n_t in enumerate(sizes):
                        if mode == "uneven":
                            pool = big_pool if i == 0 else bigB_pool
                        else:
                            pool = big_pool if n_t == big_size else bigB_pool
                        slab = pool.tile([128, n_t * H], f32,
                                         tag=f"blk{n_t}")
                        s3 = slab[:].rearrange("p (n h) -> p n h", n=n_t)
                        src = enc.ap()[128 * o:128 * (o + n_t), :].rearrange(
                            "(p n) h -> p (n h)", n=n_t)
                        q = (queues[min(i, 1) % len(queues)]
                             if mode == "uneven"
                             else queues[i % len(queues)])
                        do_slab(slab, s3, src, n_t, o, q)
                        o += n_t
                else:
                    T = NBLK // n
                    enc_v = enc.ap().rearrange("(t p n) h -> t p (n h)",
                                               p=128, n=n)
                    for t in range(T):
                        slab = big_pool.tile([128, n * H], f32, tag="blk")
                        s3 = slab[:].rearrange("p (n h) -> p n h", n=n)
                        do_slab(slab, s3, enc_v[t], n, t * n,
                                queues[t % len(queues)])

            nc.sync.dma_start(eng_out[:], e_sb[:])

    nc.compile()
    return nc


def _get_program():
    global BUILD_KW
    key = ("nc", tuple(sorted(BUILD_KW.items())))
    if key not in _CACHE:
        try:
            _CACHE[key] = _build_program(**BUILD_KW)
        except Exception:
            # safety net: stock-ops fallback (no custom DVE op) — slower
            # (~140us vs ~95us warm) but uses only standard instructions
            BUILD_KW = {"n": 16, "bufs": 6, "queues": ("sync", "scalar"),
                        "mode": "ttr", "w_via_pe": True}
            key = ("nc", tuple(sorted(BUILD_KW.items())))
            if key not in _CACHE:
                _CACHE[key] = _build_program(**BUILD_KW)
    return _CACHE[key]


def kernel(hidden, encoder_outputs, _trace=False, _trace_kwargs=None):
    from concourse.bass_utils import run_bass_kernel_spmd

    nc = _get_program()
    hidden = np.ascontiguousarray(
        np.asarray(hidden, dtype=np.float32)
    ).reshape(1, H)
    enc = np.ascontiguousarray(np.asarray(encoder_outputs, dtype=np.float32))
    assert enc.shape == (SEQ, H)

    in_maps = [
        {"enc": enc[c * SHARD:(c + 1) * SHARD], "hid": hidden}
        for c in range(NCORES)
    ]
    res = run_bass_kernel_spmd(
        nc,
        in_maps,
        core_ids=list(range(NCORES)),
        trace=_trace,
        **(_trace_kwargs or {}),
    )
    _CACHE["last_results"] = res

    u = np.stack([res.results[c]["energies"] for c in range(NCORES)])
    mode = BUILD_KW["mode"]
    if mode == "uneven":
        sizes = SIZES_UNEVEN
    elif mode == "ramp":
        sizes = SIZES_RAMP
    else:
        n = BUILD_KW["n"]
        sizes = (n,) * (NBLK // n)
    # per slab: e_sb[:, o:o+n_t]; row = 128*o + p*n_t + j
    e = np.empty((NCORES, SHARD), dtype=np.float64)
    o = 0
    for n_t in sizes:
        u3 = u[:, :, o:o + n_t].astype(np.float64)  # [c, p, j]
        if mode in ("segdot", "uneven", "ramp"):
            # samples are cumulative within the slab: adjacent diffs
            # recover the per-row energies
            u3 = np.diff(
                np.concatenate([np.zeros((NCORES, 128, 1)), u3], axis=2),
                axis=2,
            )
        e[:, 128 * o:128 * (o + n_t)] = u3.reshape(NCORES, 128 * n_t)
        o += n_t
    e = e.reshape(-1)

    e -= e.max()
    p = np.exp(e)
    p /= p.sum()
    return p.reshape(1, 1, SEQ).astype(np.float32)


# revision 22
# speedup vs baseline: 1.2344x; 1.2344x over previous
"""Trainium2 Bass kernel for nn_Attn_17059610099812.

reference:
    energies = einsum('sh,h->s', encoder_outputs[131072, 512], hidden[512])
    attn = softmax(energies)   -> [1, 1, 131072]

Strategy (8 NeuronCores, SPMD):
  - Shard encoder_outputs along seq_len: 16384 rows per core (host-side
    split via per-core input maps).
  - Per core: stream the 32MB shard in 4 slabs of 8MB (pn layout: each SBUF
    partition holds n=32 consecutive rows as one contiguous 64KB DMA
    descriptor), double-buffered across 2 HWDGE queues; DMA runs at ~430GB/s.
  - mode "segdot": per slab, ONE single-pass custom DVE instruction
    (multiply-cumsum: out[k] = sum_{i<=k} slab[i]*w[i], fp32 scan at
    1 elem/partition/cycle) computes running dot products in-place; a tiny
    strided copy samples column 511 of each 512-element page = cumulative
    row energies. The host recovers per-row energies by adjacent
    differences. DVE does exactly one pass over the data, so the kernel
    stays DMA-bound (~80-90us/core) with ~20 instructions total.
  - mode "ttr": stock two-pass fallback (tensor_mul + segmented
    tensor_reduce) — no custom op, ~125us/core.
  - Device ships raw energy samples (64KB/core); the host finishes with a
    float64 softmax over 131072 values (negligible) and the pn unpermute.

kernel() accepts the FULL inputs and returns the FULL [1, 1, 131072] output.
"""

import numpy as np

SEQ = 131072
H = 512
NCORES = 8
SHARD = SEQ // NCORES          # 16384 rows per core
NBLK = SHARD // 128            # 128 energies per partition

_CACHE = {}

BUILD_KW = {"n": 32, "bufs": 3, "queues": ("sync", "scalar"),
            "mode": "ramp", "w_via_pe": True}


def _register_mult_cumsum():
    """Process-local custom DVE op: out[p,k] = sum_{i<=k} in0[p,i]*in1[p,i].

    The framework writes per-NEFF DVE tables from the process-local op
    catalog; registering here (instead of editing dve_ops.py) is the
    self-contained equivalent of adding the op to the catalog. uops hashes
    are pinned from lower() output, same as test_ops_golden would print.
    """
    from concourse import dve_ops
    from concourse.dve_spec import AluOp, Spec, Src0, Src1, scan
    from concourse.dve_uop import DveOpSpec

    name = "ANT_MULT_CUMSUM"
    for op in dve_ops.OPS:
        if op.name == name:
            return op

    def _ref(in0, in1, c0, c1, c2):
        a = np.asarray(in0, dtype=np.float32)
        b = np.asarray(in1, dtype=np.float32)
        if b.shape != a.shape:
            b = np.broadcast_to(
                b.reshape(b.shape[0], -1), a.reshape(a.shape[0], -1).shape
            ).reshape(a.shape)
        prod = a * b
        p2 = prod.reshape(prod.shape[0], -1)
        return np.cumsum(p2, axis=-1, dtype=np.float32).reshape(prod.shape)

    spec = Spec(body=scan(AluOp.ADD, Src0 * Src1), reference=_ref)
    row = max(dve_ops._SUB_OPCODE_FOR_NAME.values()) + 1
    assert row < 0x20
    op = dve_ops.DveOp(name, spec, subdim=False, uops_sha={})
    dve_ops.OPS.append(op)
    dve_ops.CUSTOM_DVE_SPECS[name] = spec
    dve_ops._SUB_OPCODE_FOR_NAME[name] = row
    for ver in ("v3", "v4"):
        r = DveOpSpec(name=name, opcode=row, uops=dve_ops.lower(spec, ver=ver),
                      rd1_en=dve_ops.has_src1(spec))
        op.uops_sha[ver] = r.sha(ver)
    return op


SIZES_UNEVEN = (64, 32, 32)     # rows/partition per slab; sum == NBLK
# small slab first for an early scan start; sizes interleaved so the two
# HWDGE queues carry equal bytes (sync: 16+16+32, scalar: 32+32 rows/part)
SIZES_RAMP = (16, 32, 16, 32, 32)
# queue for the 1-descriptor hid DMA: scalar, whose first slab is the big
# 32-row one — keeps the hid DMA's ~2us fixed latency off the sync ring
# head so slab0 (16 rows, the critical first scan) starts immediately
HID_QUEUE = "scalar"


def _build_program(n=32, bufs=2, queues=("sync", "scalar"), mode="segdot",
                   w_via_pe=False, _repeat=1):
    import concourse.bacc as bacc
    import concourse.mybir as mybir
    import concourse.tile as tile

    f32 = mybir.dt.float32
    Alu = mybir.AluOpType
    Ax = mybir.AxisListType

    if mode != "uneven":
        assert NBLK % n == 0
    if mode in ("segdot", "uneven", "ramp"):
        cumsum_op = _register_mult_cumsum()

    nc = bacc.Bacc(
        "TRN2", target_bir_lowering=False, debug=False, num_devices=NCORES
    )
    enc = nc.dram_tensor("enc", [SHARD, H], f32, kind="ExternalInput")
    hid = nc.dram_tensor("hid", [1, H], f32, kind="ExternalInput")
    eng_out = nc.dram_tensor("energies", [128, NBLK], f32,
                             kind="ExternalOutput")

    big_bufs = 1 if mode == "uneven" else (2 if mode == "ramp" else bufs)
    bigB_bufs = 2 if mode == "ramp" else 1
    with tile.TileContext(nc) as tc:
        with (
            tc.tile_pool(name="big", bufs=big_bufs) as big_pool,
            tc.tile_pool(name="small", bufs=1) as small_pool,
            tc.tile_pool(name="bigB", bufs=bigB_bufs) as bigB_pool,
        ):
            # w_sb[p, h] = hidden[h] on every partition
            w_sb = small_pool.tile([128, H], f32, tag="w")
            if w_via_pe:
                # ones-matmul broadcast: the hid DMA is a single descriptor
                # (vs 128 for a partition-broadcast DMA)
                with tc.tile_pool(name="psum", bufs=1, space="PSUM") as psum_p:
                    ones_t = small_pool.tile([1, 128], f32, tag="ones")
                    nc.vector.memset(ones_t[:], 1.0)
                    hid_sb = small_pool.tile([1, H], f32, tag="hid")
                    getattr(nc, HID_QUEUE).dma_start(hid_sb[:], hid[:])
                    w_ps = psum_p.tile([128, H], f32, tag="wps")
                    nc.tensor.matmul(w_ps[:], ones_t[:], hid_sb[:],
                                     start=True, stop=True)
                    nc.scalar.copy(w_sb[:], w_ps[:])
            else:
                nc.sync.dma_start(w_sb[:], hid.ap().broadcast_to([128, H]))

            # e_sb[p, o + j]: cumulative energy sample of page j in the slab
            # at block-offset o ("segdot"/"uneven"); plain energy for "ttr"
            e_sb = small_pool.tile([128, NBLK], f32, tag="e")

            def do_slab(slab, s3, src_ap, n_t, o, queue):
                w_b = w_sb[:].unsqueeze(1).broadcast_to([128, n_t, H])
                getattr(nc, queue).dma_start(slab[:], src_ap)
                if mode == "ttr":
                    nc.vector.tensor_mul(s3, s3, w_b)
                    nc.vector.tensor_reduce(
                        e_sb[:, o:o + n_t], s3, axis=Ax.X, op=Alu.add)
                else:
                    nc.vector._custom_dve(cumsum_op, out=s3, in0=s3, in1=w_b)
                    nc.vector.tensor_copy(
                        e_sb[:, o:o + n_t], s3[:, :, H - 1:H].squeeze(2))

            for rep in range(_repeat):
                if mode in ("uneven", "ramp"):
                    # uneven: slab A (big pool); the rest share bigB.
                    # ramp: small slabs (bigB) first for an early scan
                    # start, then big slabs (big pool).
                    sizes = SIZES_UNEVEN if mode == "uneven" else SIZES_RAMP
                    big_size = max(sizes)
                    o = 0
                    for i, n_t in enumerate(sizes):
                        if mode == "uneven":
                            pool = big_pool if i == 0 else bigB_pool
                        else:
                            pool = big_pool if n_t == big_size else bigB_pool
                        slab = pool.tile([128, n_t * H], f32,
                                         tag=f"blk{n_t}")
                        s3 = slab[:].rearrange("p (n h) -> p n h", n=n_t)
                        src = enc.ap()[128 * o:128 * (o + n_t), :].rearrange(
                            "(p n) h -> p (n h)", n=n_t)
                        q = (queues[min(i, 1) % len(queues)]
                             if mode == "uneven"
                             else queues[i % len(queues)])
                        do_slab(slab, s3, src, n_t, o, q)
                        o += n_t
                else:
                    T = NBLK // n
                    enc_v = enc.ap().rearrange("(t p n) h -> t p (n h)",
                                               p=128, n=n)
                    for t in range(T):
                        slab = big_pool.tile([128, n * H], f32, tag="blk")
                        s3 = slab[:].rearrange("p (n h) -> p n h", n=n)
                        do_slab(slab, s3, enc_v[t], n, t * n,
                                queues[t % len(queues)])

            nc.sync.dma_start(eng_out[:], e_sb[:])

    nc.compile()
    return nc


def _get_program():
    global BUILD_KW
    key = ("nc", tuple(sorted(BUILD_KW.items())))
    if key not in _CACHE:
        try:
            _CACHE[key] = _build_program(**BUILD_KW)
        except Exception:
            # safety net: stock-ops fallback (no custom DVE op) — slower
            # (~140us vs ~95us warm) but uses only standard instructions
            BUILD_KW = {"n": 16, "bufs": 6, "queues": ("sync", "scalar"),
                        "mode": "ttr", "w_via_pe": True}
            key = ("nc", tuple(sorted(BUILD_KW.items())))
            if key not in _CACHE:
                _CACHE[key] = _build_program(**BUILD_KW)
    return _CACHE[key]


def kernel(hidden, encoder_outputs, _trace=False, _trace_kwargs=None):
    from concourse.bass_utils import run_bass_kernel_spmd

    nc = _get_program()
    hidden = np.ascontiguousarray(
        np.asarray(hidden, dtype=np.float32)
    ).reshape(1, H)
    enc = np.ascontiguousarray(np.asarray(encoder_outputs, dtype=np.float32))
    assert enc.shape == (SEQ, H)

    in_maps = [
        {"enc": enc[c * SHARD:(c + 1) * SHARD], "hid": hidden}
        for c in range(NCORES)
    ]
    res = run_bass_kernel_spmd(
        nc,
        in_maps,
        core_ids=list(range(NCORES)),
        trace=_trace,
        **(_trace_kwargs or {}),
    )
    _CACHE["last_results"] = res

    u = np.stack([res.results[c]["energies"] for c in range(NCORES)])
    mode = BUILD_KW["mode"]
    if mode == "uneven":
        sizes = SIZES_UNEVEN
    elif mode == "ramp":
        sizes = SIZES_RAMP
    else:
        n = BUILD_KW["n"]
        sizes = (n,) * (NBLK // n)
    # per slab: e_sb[:, o:o+n_t]; row = 128*o + p*n_t + j
    e = np.empty((NCORES, SHARD), dtype=np.float64)
    o = 0
    for n_t in sizes:
        u3 = u[:, :, o:o + n_t].astype(np.float64)  # [c, p, j]
        if mode in ("segdot", "uneven", "ramp"):
            # samples are cumulative within the slab: adjacent diffs
            # recover the per-row energies
            u3 = np.diff(
                np.concatenate([np.zeros((NCORES, 128, 1)), u3], axis=2),
                axis=2,
            )
        e[:, 128 * o:128 * (o + n_t)] = u3.reshape(NCORES, 128 * n_t)
        o += n_t
    e = e.reshape(-1)

    e -= e.max()
    p = np.exp(e)
    p /= p.sum()
    return p.reshape(1, 1, SEQ).astype(np.float32)
